# revision 1
# baseline (speedup 1.0000x reference)
"""ChunkFlowClassifier Trainium2 kernel.

Math (per sample, reference.py):
  L = sum(attention_mask); mid = L // 2
  first_pool  = mean(hidden[1:mid])        # [H]
  second_pool = mean(hidden[mid:L-1])      # [H]
  fh, sh = LN(first_pool), LN(second_pool)
  flow = [fh, sh, sh - fh]                 # [3H]
  out = gelu(gelu(flow @ W1 + b1) @ W2 + b2) @ W3 + b3   # [5]

Strategy: data-parallel over 8 NeuronCores (8 samples/core). Host packs
only the rows each sample actually uses (positions 1..L-2; lengths are
ragged, avg ~50% of S) into a dense fp16 buffer plus per-row 0/1 mask
columns that route each row into one of 16 (sample, half) accumulators.
The device streams the packed buffer and pools via PE matmuls
  psum[16, H] += mask_tile[128, 16].T @ x_tile[128, H]
then runs LayerNorm + the MLP on-chip once per core.

Host-side algebraic folds (exact, just reassociation):
  flow @ W1 = fh@(W1a - W1c) + sh@(W1b + W1c)        (W1 = [W1a; W1b; W1c])
  LN scale/shift:  (xhat*g + b) @ M = xhat @ (g[:,None]*M) + b @ M
so the device only needs xhat (plain normalize) and a folded
W1f[2H, 512] (fp16) + b1f[512].
"""

import numpy as np

B, S, H = 64, 2048, 768
NCORES = 8
SPC = 8            # samples per core
C = 2              # 128-row tiles per DMA chunk
XBUFS = 16          # SBUF double-buffering depth for the stream
ALT_ENGINE = True  # alternate x-chunk DMAs between the two HWDGE rings
XDT_NAME = "float16"  # dtype hidden is streamed in

_NC_CACHE = {}


def _build_nc(nchunk, repeat=1):
    """Build + compile the per-core Bass program for `nchunk` C-tile chunks.

    repeat > 1 wraps the streaming loop in a Tile For_i that re-streams the
    same data `repeat` times (used only for timing; output is unchanged).
    """
    import concourse.bacc as bacc
    import concourse.tile as tile
    from concourse import mybir

    dt = mybir.dt
    f32 = dt.float32
    xdt = getattr(dt, XDT_NAME)
    Alu = mybir.AluOpType
    Act = mybir.ActivationFunctionType

    NT = nchunk * C

    nc = bacc.Bacc("TRN2", target_bir_lowering=False, debug=False,
                   num_devices=NCORES)

    def din(name, shape, d=f32):
        return nc.dram_tensor(name, shape, d, kind="ExternalInput").ap()

    xin = din("xin", [nchunk, 128, C * H], xdt)
    mc = din("mc", [128, NT * 16], xdt)
    epsc = din("epsc", [16, 1])
    idn = din("idn", [16, 16])
    w1 = din("w1", [2 * H, 512], xdt)       # folded (see module docstring)
    b1 = din("b1", [1, 512], xdt)
    w2 = din("w2", [512, 128])
    b2 = din("b2", [1, 128])
    w3 = din("w3", [128, 5])
    b3 = din("b3", [1, 5])
    out = nc.dram_tensor("out", [SPC, 5], f32, kind="ExternalOutput").ap()

    with tile.TileContext(nc) as tc:
        with (
            tc.tile_pool(name="xp", bufs=XBUFS) as xp,
            tc.tile_pool(name="sg", bufs=1) as sg,
            tc.tile_pool(name="sm", bufs=1) as sm,
            tc.tile_pool(name="tpp", bufs=2, space="PSUM") as tpp,
            tc.tile_pool(name="mlp", bufs=1, space="PSUM") as mlp,
            tc.tile_pool(name="acc", bufs=1, space="PSUM") as acc,
        ):
            mc_sb = sg.tile([128, NT * 16], xdt)
            nc.sync.dma_start(out=mc_sb, in_=mc)
            # weights/constants: small now, prefetch alongside the stream
            w1_sb = sg.tile([128, 12, 512], xdt)
            nc.scalar.dma_start(out=w1_sb, in_=w1.rearrange("(k p) n -> p k n", p=128))
            w2_sb = sg.tile([128, 4, 128], f32)
            nc.scalar.dma_start(out=w2_sb, in_=w2.rearrange("(k p) n -> p k n", p=128))
            w3_sb = sg.tile([128, 5], f32)
            nc.scalar.dma_start(out=w3_sb, in_=w3)
            b1_sb = sm.tile([1, 512], xdt)
            nc.scalar.dma_start(out=b1_sb, in_=b1)
            b2_sb = sm.tile([1, 128], f32)
            nc.scalar.dma_start(out=b2_sb, in_=b2)
            b3_sb = sm.tile([1, 5], f32)
            nc.scalar.dma_start(out=b3_sb, in_=b3)
            epsc_sb = sm.tile([16, 1], f32)
            nc.sync.dma_start(out=epsc_sb, in_=epsc)
            idn_sb = sm.tile([16, 16], f32)
            nc.sync.dma_start(out=idn_sb, in_=idn)
            ones_sb = sm.tile([1, SPC], xdt)
            nc.vector.memset(ones_sb, 1.0)
            onesf_sb = sm.tile([1, SPC], f32)
            nc.vector.memset(onesf_sb, 1.0)
            scr_sb = sm.tile([1, 2], f32)
            nc.vector.memset(scr_sb, 1.0)
            # touch Sqrt+Gelu once early so ACT table loads overlap the stream
            nc.scalar.activation(out=scr_sb[:, 0:1], in_=scr_sb[:, 0:1],
                                 func=Act.Sqrt)
            nc.scalar.activation(out=scr_sb[:, 1:2], in_=scr_sb[:, 1:2],
                                 func=Act.Gelu)

            ps1 = acc.tile([16, 512], f32)
            ps2 = acc.tile([16, 256], f32)

            def stream_body(_i=None):
                for g in range(nchunk):
                    xt = xp.tile([128, C * H], xdt, tag="x")
                    eng = nc.sync if (g % 2 == 0 or not ALT_ENGINE) else nc.scalar
                    eng.dma_start(out=xt, in_=xin[g])
                    for c in range(C):
                        t = g * C + c
                        first = t == 0
                        last = t == NT - 1
                        lhs = mc_sb[:, t * 16:(t + 1) * 16]
                        nc.tensor.matmul(ps1, lhs, xt[:, c * H:c * H + 512],
                                         start=first, stop=last)
                        nc.tensor.matmul(ps2, lhs, xt[:, c * H + 512:(c + 1) * H],
                                         start=first, stop=last)

            def full_pass():
                stream_body()
                # LayerNorm directly on the raw sums: LN is scale-invariant,
                # with eps scaled by cnt^2 (host-provided) to stay exact.
                stats = sm.tile([16, 3, 6], f32)
                nc.vector.bn_stats(out=stats[:, 0, :], in_=ps1[:, 0:256])
                nc.vector.bn_stats(out=stats[:, 1, :], in_=ps1[:, 256:512])
                nc.vector.bn_stats(out=stats[:, 2, :], in_=ps2)
                mv = sm.tile([16, 2], f32)
                nc.vector.bn_aggr(out=mv, in_=stats)
                rstd = sm.tile([16, 1], f32)
                nc.scalar.activation(out=rstd, in_=mv[:, 1:2], func=Act.Sqrt,
                                     bias=epsc_sb, scale=1.0)
                nc.vector.reciprocal(out=rstd, in_=rstd)
                # keep PE busy through the LN chain so HAM stays at full clock
                warm = mlp.tile([16, 32], f32, tag="warm")
                nc.tensor.matmul(warm[:, 0:18], idn_sb, stats.rearrange("p a b -> p (a b)"),
                                 start=True, stop=True)
                nc.tensor.matmul(warm[:, 18:20], idn_sb, mv, start=True, stop=True)
                nc.tensor.matmul(warm[:, 20:21], idn_sb, rstd, start=True, stop=True)
                xn1 = sg.tile([16, 512], f32)
                xn2 = sg.tile([16, 256], f32)
                nc.vector.tensor_scalar(out=xn1, in0=ps1, scalar1=mv[:, 0:1],
                                        scalar2=rstd, op0=Alu.subtract, op1=Alu.mult)
                nc.vector.tensor_scalar(out=xn2, in0=ps2, scalar1=mv[:, 0:1],
                                        scalar2=rstd, op0=Alu.subtract, op1=Alu.mult)

                # transpose the 16 normalized vectors -> 12 k-tiles [128, 8] fp16
                flowT = sg.tile([128, 12, SPC], xdt)
                tp6 = tpp.tile([128, 6, 16], f32, tag="tp")
                for c6 in range(6):
                    src_ap = (xn1[:, c6 * 128:(c6 + 1) * 128] if c6 < 4
                              else xn2[:, (c6 - 4) * 128:(c6 - 3) * 128])
                    nc.tensor.matmul(tp6[:, c6, :], src_ap,
                                     idn_sb, start=True, stop=True)
                # tp6[:, c, h*8:h*8+8] holds (half h, chunk c); flowT k-tile
                # order is [fh chunks 0..5 | sh chunks 0..5]
                nc.vector.tensor_copy(flowT[:, 0:6, :], tp6[:, :, 0:SPC])
                nc.vector.tensor_copy(flowT[:, 6:12, :], tp6[:, :, SPC:16])

                # layer 1: h1[8, 512] = gelu(fh @ W1f[:H] + sh @ W1f[H:] + b1f)
                h1ps = mlp.tile([SPC, 512], f32, tag="h1")
                for k in range(12):
                    nc.tensor.matmul(h1ps, flowT[:, k, :], w1_sb[:, k, :],
                                     start=(k == 0), stop=False)
                nc.tensor.matmul(h1ps, ones_sb, b1_sb, start=False, stop=True)
                h1 = sg.tile([SPC, 512], f32)
                nc.scalar.activation(out=h1, in_=h1ps, func=Act.Gelu)

                h1T = sg.tile([128, 4, SPC], f32)
                tp4 = tpp.tile([128, 4, SPC], f32, tag="tp")
                for k in range(4):
                    nc.tensor.matmul(tp4[:, k, :], h1[:, k * 128:(k + 1) * 128],
                                     idn_sb[0:SPC, 0:SPC], start=True, stop=True)
                nc.vector.tensor_copy(h1T, tp4)

                # layer 2: h2[8, 128] = gelu(h1 @ W2 + b2)
                h2ps = mlp.tile([SPC, 128], f32, tag="h2")
                for k in range(4):
                    nc.tensor.matmul(h2ps, h1T[:, k, :], w2_sb[:, k, :],
                                     start=(k == 0), stop=False)
                nc.tensor.matmul(h2ps, onesf_sb, b2_sb, start=False, stop=True)
                h2 = sg.tile([SPC, 128], f32)
                nc.scalar.activation(out=h2, in_=h2ps, func=Act.Gelu)

                tp = tpp.tile([128, 16], f32, tag="tp")
                nc.tensor.matmul(tp[:, 0:SPC], h2, idn_sb[0:SPC, 0:SPC],
                                 start=True, stop=True)
                h2T = sg.tile([128, SPC], f32)
                nc.vector.tensor_copy(h2T, tp[:, 0:SPC])

                # layer 3: out[8, 5] = h2 @ W3 + b3
                ops = mlp.tile([SPC, 5], f32, tag="o")
                nc.tensor.matmul(ops, h2T, w3_sb, start=True, stop=False)
                nc.tensor.matmul(ops, onesf_sb, b3_sb, start=False, stop=True)
                o_sb = sm.tile([SPC, 5], f32)
                nc.vector.tensor_copy(o_sb, ops)
                nc.sync.dma_start(out=out, in_=o_sb)

            if repeat == 1:
                full_pass()
            else:
                unroll = globals().get("_TIMING_UNROLL", 1)
                with tc.For_i(0, repeat, 1) as _i:
                    for _u in range(unroll):
                        full_pass()

    nc.compile()
    return nc


def _get_nc(nchunk, repeat=1):
    key = (nchunk, repeat)
    if key not in _NC_CACHE:
        _NC_CACHE[key] = _build_nc(nchunk, repeat)
    return _NC_CACHE[key]


def _prepare(hidden, attention_mask, gamma, beta, W1, b1, W2, b2, W3, b3):
    """Host-side sharding + packing. Returns (in_maps, core_samples, nchunk)."""
    xdt = np.dtype(XDT_NAME)
    L = attention_mask.astype(np.int64).sum(1)          # [B]
    mid = L // 2
    rows = L - 2                                        # used rows per sample

    # balance total rows across cores (greedy LPT, exactly SPC samples/core)
    order = np.argsort(-rows, kind="stable")
    core_rows = [0] * NCORES
    core_samples = [[] for _ in range(NCORES)]
    for b in order:
        cands = sorted(range(NCORES),
                       key=lambda cc: (len(core_samples[cc]) >= SPC, core_rows[cc]))
        cc = cands[0]
        core_samples[cc].append(int(b))
        core_rows[cc] += int(rows[b])

    maxrows = max(core_rows)
    nchunk = max(1, -(-maxrows // (128 * C)))
    NT = nchunk * C
    R = NT * 128

    hidden2d = np.ascontiguousarray(hidden).reshape(B * S, H)
    gamma = np.asarray(gamma, np.float64)
    beta = np.asarray(beta, np.float64)
    W1 = np.asarray(W1, np.float64)
    b1 = np.asarray(b1, np.float64)
    W1a, W1b, W1c = W1[0:H], W1[H:2 * H], W1[2 * H:3 * H]
    W1f = np.concatenate([gamma[:, None] * (W1a - W1c),
                          gamma[:, None] * (W1b + W1c)], axis=0)
    b1f = b1 + beta @ (W1a + W1b)
    shared = dict(
        idn=np.eye(16, dtype=np.float32),
        w1=W1f.astype(xdt),
        b1=b1f.astype(xdt).reshape(1, -1),
        w2=np.ascontiguousarray(W2, np.float32),
        b2=np.ascontiguousarray(b2, np.float32).reshape(1, -1),
        w3=np.ascontiguousarray(W3, np.float32),
        b3=np.ascontiguousarray(b3, np.float32).reshape(1, -1),
    )

    in_maps = []
    for cc in range(NCORES):
        samples = core_samples[cc]
        rcounts = [int(rows[b]) for b in samples]
        Rc = sum(rcounts)
        idx = np.concatenate([b * S + np.arange(1, int(L[b]) - 1) for b in samples])
        packed = np.zeros((R, H), xdt)
        packed[:Rc] = hidden2d[idx]
        xin = np.ascontiguousarray(
            packed.reshape(nchunk, C, 128, H).transpose(0, 2, 1, 3)
            .reshape(nchunk, 128, C * H))

        pos = np.concatenate([np.arange(1, int(L[b]) - 1) for b in samples])
        sj = np.repeat(np.arange(SPC), rcounts)
        mids = np.repeat([int(mid[b]) for b in samples], rcounts)
        col = np.where(pos < mids, sj, sj + SPC)
        m = np.zeros((R, 16), xdt)
        m[np.arange(Rc), col] = 1.0
        mc = np.ascontiguousarray(
            m.reshape(NT, 128, 16).transpose(1, 0, 2).reshape(128, NT * 16))

        cnt1 = np.array([max(int(mid[b]) - 1, 1) for b in samples], np.float64)
        cnt2 = np.array([max(int(L[b]) - 1 - int(mid[b]), 1) for b in samples],
                        np.float64)
        epsc = np.concatenate([1e-5 * cnt1 ** 2, 1e-5 * cnt2 ** 2])
        epsc = epsc.astype(np.float32).reshape(16, 1)

        in_maps.append(dict(xin=xin, mc=mc, epsc=epsc, **shared))
    return in_maps, core_samples, nchunk


def kernel(**inputs):
    from concourse.bass_utils import run_bass_kernel_spmd

    args = {k: np.asarray(v) for k, v in inputs.items()}
    in_maps, core_samples, nchunk = _prepare(
        args["hidden"].astype(np.float32, copy=False),
        args["attention_mask"],
        args["gamma"], args["beta"],
        args["W1"], args["b1"], args["W2"], args["b2"], args["W3"], args["b3"],
    )
    nc = _get_nc(nchunk)
    res = run_bass_kernel_spmd(nc, in_maps, core_ids=list(range(NCORES)))
    out = np.zeros((B, 5), np.float32)
    for cc in range(NCORES):
        o = res.results[cc]["out"]
        for j, b in enumerate(core_samples[cc]):
            out[b] = o[j]
    return out

